# revision 1
# baseline (speedup 1.0000x reference)
"""BinsChamferLoss Trainium2 kernel.

Problem: bins (2,257,4,4), target_depth_maps (2,1,448,448).
L = 16 patches/image of 112x112 depth points, P = 256 bin centers.
For each of the 32 (n,l) patches: pairwise 1-D squared distances between
256 centers and 12544 points, min over each direction, masked mean.

Sharding: data-parallel over the 32 flattened patches -> 4 patches/core
across 8 cores.  Per core the device computes, per patch:
    A = sum_i  min_j (c_i - p~_j)^2          (p~ = p if p>0 else BIGP)
    B = sum_j  m_j * min_i (c_i - p_j)^2
    C = sum_j  m_j
The host combines: per_patch = (C>0) ? A/256 + B/max(C,1) : 0 ; mean.
(Invalid points replaced by BIGP=200 give distances ~4e4 which never win
a min when a valid point exists -- finite in fp16 -- and the all-invalid
case is gated by C>0, matching the reference's BIG masking semantics.)

Engines: ACT generates distance tiles via Square(c - p~_j) computed in
fp32 and rounded once to fp16 (bias = per-partition -p~), DVE does the
min-reduce (nearest center per point, with fp16 2x-rate half-folds
before the 1x reduce) + fp16 2x min-accumulate (nearest point per
center), PE does broadcast / transpose / partition sums.  Measured
~142 us/core on HW (loop-differenced), rel err ~1e-6.
"""

import os
from contextlib import ExitStack

import numpy as np

KP = 112               # patch side  (nn.Unfold kernel/stride)
Q = KP * KP            # 12544 points per patch
NPART = 128            # points per partition-tile
NT = Q // NPART        # 98 point columns  (12544 = 128*98)
GRP = 7                # gen columns per DVE group (98 = 14*7)
NGRP = NT // GRP       # 14 groups
PC = 256               # bin centers per patch (B-1)
BIGP = 200.0           # stand-in value for invalid (p<=0) points:
                       # (c - 200)^2 ~ 4e4 stays finite in fp16 yet is
                       # >250x any valid randn distance, so it never wins
                       # a min when a valid point exists; the all-invalid
                       # case is gated by counts>0 on the host.
ACC_INIT = 3.0e38      # (unused when first group copies)

N_CORES = 8
PATCHES_PER_CORE = 4   # 32 patches / 8 cores


def _build_module(reps=1, loop_n=None, use_f16=True, big_bufs=True, dual_acc=True, pe_groups=0, gen_from_psum=False):
    import concourse.bass as bass
    import concourse.tile as tile
    from concourse import bacc, mybir
    from concourse.masks import make_identity

    f32 = mybir.dt.float32
    f16 = mybir.dt.float16 if use_f16 else mybir.dt.float32
    Alu = mybir.AluOpType
    Act = mybir.ActivationFunctionType

    nc = bacc.Bacc("TRN2", target_bir_lowering=False, debug=False,
                   num_devices=N_CORES)

    bins4 = nc.dram_tensor("bins4", (257, PATCHES_PER_CORE), f32,
                           kind="ExternalInput").ap()
    pts = nc.dram_tensor("pts", (PATCHES_PER_CORE, NPART, NT), f32,
                         kind="ExternalInput").ap()
    outv = nc.dram_tensor("outv", (1, 3 * PATCHES_PER_CORE), f32,
                          kind="ExternalOutput").ap()

    with tile.TileContext(nc) as tc, ExitStack() as ctx:
        const_pool = ctx.enter_context(tc.tile_pool(name="const", bufs=1))
        bins_pool = ctx.enter_context(tc.tile_pool(name="bins", bufs=1))
        prep_pool = ctx.enter_context(tc.tile_pool(name="prep", bufs=3 if big_bufs else 2))
        cb_pool = ctx.enter_context(tc.tile_pool(name="cb", bufs=3 if big_bufs else 2))
        dbuf_pool = ctx.enter_context(tc.tile_pool(name="dbuf", bufs=4 if big_bufs else 3))
        acc_pool = ctx.enter_context(tc.tile_pool(name="acc", bufs=2))
        mrg_pool = ctx.enter_context(tc.tile_pool(name="mrg", bufs=3 if big_bufs else 2))
        res_pool = ctx.enter_context(tc.tile_pool(name="res", bufs=1))

        ps_bc = ctx.enter_context(tc.tile_pool(name="ps_bc",
                                                bufs=1 if pe_groups else 2,
                                                space="PSUM"))
        ps_tr = ctx.enter_context(tc.tile_pool(name="ps_tr", bufs=1, space="PSUM"))
        ps_sum = ctx.enter_context(tc.tile_pool(name="ps_sum", bufs=1, space="PSUM"))
        ps_pe = (ctx.enter_context(tc.tile_pool(name="ps_pe", bufs=2, space="PSUM"))
                 if pe_groups else None)
        stk_pool = (ctx.enter_context(tc.tile_pool(name="stk", bufs=1))
                    if pe_groups else None)

        # ---- constants -------------------------------------------------
        ident = const_pool.tile([128, 128], f32)
        make_identity(nc, ident[:])
        ident16 = ident
        if use_f16:
            ident16 = const_pool.tile([128, 128], f16)
            make_identity(nc, ident16[:])
        ones_row = const_pool.tile([1, 128], f32)      # lhsT for broadcast
        nc.vector.memset(ones_row[:], 1.0)
        ones_col = const_pool.tile([128, 1], f32)      # rhs for partition sums
        nc.vector.memset(ones_col[:], 1.0)

        # ---- centers ---------------------------------------------------
        # centers[b] = 0.5*(bins[b] + bins[b+1]) for b in 0..255, per patch.
        b_lo0 = bins_pool.tile([128, PATCHES_PER_CORE], f32, tag="b0")
        b_lo1 = bins_pool.tile([128, PATCHES_PER_CORE], f32, tag="b1")
        b_hi0 = bins_pool.tile([128, PATCHES_PER_CORE], f32, tag="b2")
        b_hi1 = bins_pool.tile([128, PATCHES_PER_CORE], f32, tag="b3")
        nc.sync.dma_start(b_lo0[:], bins4[0:128, :])
        nc.sync.dma_start(b_lo1[:], bins4[1:129, :])
        nc.sync.dma_start(b_hi0[:], bins4[128:256, :])
        nc.sync.dma_start(b_hi1[:], bins4[129:257, :])
        ch0 = bins_pool.tile([128, PATCHES_PER_CORE], f32, tag="ch0")
        ch1 = bins_pool.tile([128, PATCHES_PER_CORE], f32, tag="ch1")
        nc.vector.tensor_add(ch0[:], b_lo0[:], b_lo1[:])
        nc.vector.tensor_scalar_mul(ch0[:], ch0[:], 0.5)
        nc.vector.tensor_add(ch1[:], b_hi0[:], b_hi1[:])
        nc.vector.tensor_scalar_mul(ch1[:], ch1[:], 0.5)

        # transpose -> cT (4, 256): row k = patch k's 256 centers
        pt0 = ps_tr.tile([PATCHES_PER_CORE, 128], f32, tag="trA")
        pt1 = ps_tr.tile([PATCHES_PER_CORE, 128], f32, tag="trB")
        nc.tensor.transpose(pt0[:], ch0[:], ident[:])
        nc.tensor.transpose(pt1[:], ch1[:], ident[:])
        cT = bins_pool.tile([PATCHES_PER_CORE, PC], f32, tag="cT")
        nc.vector.tensor_copy(cT[:, 0:128], pt0[:])
        nc.vector.tensor_copy(cT[:, 128:256], pt1[:])
        # flatten the 4 center rows onto partition 0 so they can feed the
        # broadcast matmul (contraction operands must share base partition 0)
        cflat = bins_pool.tile([1, PATCHES_PER_CORE * PC], f32, tag="cflat")
        nc.sync.dma_start(cflat[:], cT[:])

        stacked = crow2 = None
        if pe_groups:
            # lhsT rows for the PE diff matmuls: row0 = -p~ (per patch, via
            # transpose + flatten DMA), row1 = ones (set once)
            stacked = stk_pool.tile([2, Q], f32, tag="stk")
            nc.vector.memset(stacked[:], 1.0)   # row 0 re-DMA'd per patch
            # rhs rows: row0 = +1s (once), row1 = centers of patch k (DMA)
            crow2 = stk_pool.tile([2, PC], f32, tag="crow2")
            nc.vector.memset(crow2[0:1, :], 1.0)

        results = res_pool.tile([1, 3 * PATCHES_PER_CORE], f32)

        loop_ctx = (tc.For_i(0, loop_n, 1,
                             hint_engines=(mybir.EngineType.Activation,
                                           mybir.EngineType.DVE))
                    if loop_n is not None else None)
        if loop_ctx is not None:
            ctx.enter_context(loop_ctx)

        for k in [k for _ in range(reps) for k in range(PATCHES_PER_CORE)]:
            # ---- broadcast centers of patch k to all partitions --------
            pb = ps_bc.tile([NPART, PC], f32, tag="pb")
            nc.tensor.matmul(pb[:], ones_row[:], cflat[:, k * PC:(k + 1) * PC],
                             start=True, stop=True)
            if gen_from_psum:
                cbc = pb          # ACT reads the broadcast straight from PSUM
            else:
                cbc = cb_pool.tile([NPART, PC], f32, tag="cbc")
                nc.scalar.copy(cbc[:], pb[:])

            # ---- load + prep points ------------------------------------
            p0 = prep_pool.tile([NPART, NT], f32, tag="p0")
            nc.sync.dma_start(p0[:], pts[k])
            msk = prep_pool.tile([NPART, NT], f32, tag="msk")
            nc.vector.tensor_scalar(msk[:], p0[:], 0.0, None, op0=Alu.is_gt)
            inv = prep_pool.tile([NPART, NT], f32, tag="inv")
            nc.vector.tensor_scalar(inv[:], p0[:], 0.0, None, op0=Alu.is_le)
            # npt = -p for valid points, ~-1e6 for invalid: (inv*-BIGP) - p
            npt = prep_pool.tile([NPART, NT], f32, tag="npt")
            nc.vector.scalar_tensor_tensor(npt[:], inv[:], -BIGP, p0[:],
                                           op0=Alu.mult, op1=Alu.subtract)
            ccol = prep_pool.tile([NPART, 1], f32, tag="ccol")
            nc.vector.tensor_reduce(ccol[:], msk[:], axis=mybir.AxisListType.X,
                                    op=Alu.add)

            if pe_groups:
                ptT_ps = ps_tr.tile([NT, 128], f32, tag="trA")
                nc.tensor.transpose(ptT_ps[:], npt[:], ident[:])
                ptT_sb = prep_pool.tile([NT, 128], f32, tag="ptT")
                nc.vector.tensor_copy(ptT_sb[:], ptT_ps[:])
                nc.sync.dma_start(stacked[0:1, :], ptT_sb[:])
                nc.sync.dma_start(crow2[1:2, :], cflat[:, k * PC:(k + 1) * PC])

            minx = prep_pool.tile([NPART, NT], f32, tag="minx")
            acc = acc_pool.tile([NPART, GRP * PC], f16, tag="acc")
            accO = None
            if dual_acc:
                accO = acc_pool.tile([NPART, GRP * PC], f16, tag="accO")

            # ---- main loop: distance tiles + two min streams -----------
            for g in range(NGRP):
                dbuf = dbuf_pool.tile([NPART, GRP * PC], f16, tag="dbuf")
                if g >= NGRP - pe_groups:
                    # PE path: exact fp32 diffs (c - p~) via K=2 matmuls,
                    # squared on ACT two columns at a time from PSUM
                    for pair in range(4):
                        ncols = 2 if pair < 3 else 1
                        psd = ps_pe.tile([NPART, 512], f32, tag="psd")
                        for h in range(ncols):
                            col = g * GRP + 2 * pair + h
                            nc.tensor.matmul(
                                psd[:, 256 * h:256 * (h + 1)],
                                stacked[0:2, 128 * col:128 * (col + 1)],
                                crow2[:], start=True, stop=True)
                        nc.scalar.activation(
                            dbuf[:, 2 * pair * PC:(2 * pair + ncols) * PC],
                            psd[:, 0:ncols * PC], Act.Square,
                            bias=0.0, scale=1.0)
                else:
                    for t in range(GRP):
                        col = g * GRP + t
                        nc.scalar.activation(
                            dbuf[:, t * PC:(t + 1) * PC], cbc[:], Act.Square,
                            bias=npt[:, col:col + 1], scale=1.0)
                # nearest-center distances for these 7*128 points:
                # two half-folds at fp16 2x rate, then a 1x reduce
                v4 = dbuf[:].rearrange("p (g two h) -> p g two h",
                                       two=2, h=PC // 2)
                u1 = mrg_pool.tile([NPART, GRP * (PC // 2)], f16, tag="u1")
                nc.vector.tensor_tensor(
                    u1[:].rearrange("p (g h) -> p g h", h=PC // 2),
                    v4[:, :, 0, :], v4[:, :, 1, :], op=Alu.min)
                w4 = u1[:].rearrange("p (g two h) -> p g two h",
                                     two=2, h=PC // 4)
                u2 = mrg_pool.tile([NPART, GRP * (PC // 4)], f16, tag="u2")
                nc.vector.tensor_tensor(
                    u2[:].rearrange("p (g h) -> p g h", h=PC // 4),
                    w4[:, :, 0, :], w4[:, :, 1, :], op=Alu.min)
                nc.vector.tensor_reduce(
                    minx[:, g * GRP:(g + 1) * GRP],
                    u2[:].rearrange("p (g c) -> p g c", c=PC // 4),
                    axis=mybir.AxisListType.X, op=Alu.min)
                # accumulate nearest-point distances per center; with
                # dual_acc, even/odd groups use separate accumulators so
                # consecutive min-accumulates are independent (no RAW chain)
                tgt = acc if (not dual_acc or g % 2 == 0) else accO
                if g < 2:
                    nc.vector.tensor_copy(tgt[:], dbuf[:])
                else:
                    nc.vector.tensor_tensor(tgt[:], tgt[:], dbuf[:],
                                            op=Alu.min)

            # ---- fold the 7 column blocks of acc to one (128,256) ------
            if dual_acc:
                nc.vector.tensor_tensor(acc[:], acc[:], accO[:], op=Alu.min)
            m4 = mrg_pool.tile([NPART, 3 * PC], f16, tag="m4")
            nc.vector.tensor_tensor(m4[:], acc[:, 0:3 * PC],
                                    acc[:, 3 * PC:6 * PC], op=Alu.min)
            m2 = mrg_pool.tile([NPART, 2 * PC], f16, tag="m2")
            nc.vector.tensor_tensor(m2[:], m4[:, 0:2 * PC],
                                    m4[:, PC:3 * PC], op=Alu.min)
            accf = mrg_pool.tile([NPART, PC], f16, tag="accf")
            nc.vector.tensor_tensor(accf[:], m2[:, 0:PC], m2[:, PC:2 * PC],
                                    op=Alu.min)
            nc.vector.tensor_tensor(accf[:], accf[:], acc[:, 6 * PC:7 * PC],
                                    op=Alu.min)

            # ---- min over points (partitions) via transpose ------------
            trA = ps_tr.tile([128, 128], f16, tag="trA")
            nc.tensor.transpose(trA[:], accf[:, 0:128], ident16[:])
            miny0 = mrg_pool.tile([128, 1], f32, tag="miny0")
            nc.vector.tensor_reduce(miny0[:], trA[:], axis=mybir.AxisListType.X,
                                    op=Alu.min)
            trB = ps_tr.tile([128, 128], f16, tag="trB")
            nc.tensor.transpose(trB[:], accf[:, 128:256], ident16[:])
            miny1 = mrg_pool.tile([128, 1], f32, tag="miny1")
            nc.vector.tensor_reduce(miny1[:], trB[:], axis=mybir.AxisListType.X,
                                    op=Alu.min)

            # A = sum_i min_y[i]
            psA = ps_sum.tile([1, 1], f32, tag="psA")
            nc.tensor.matmul(psA[:], miny0[:], ones_col[:], start=True, stop=False)
            nc.tensor.matmul(psA[:], miny1[:], ones_col[:], start=False, stop=True)

            # B = sum_j m_j * minx_j
            w = prep_pool.tile([NPART, NT], f32, tag="w")
            nc.vector.tensor_tensor(w[:], minx[:], msk[:], op=Alu.mult)
            bcol = prep_pool.tile([NPART, 1], f32, tag="bcol")
            nc.vector.tensor_reduce(bcol[:], w[:], axis=mybir.AxisListType.X,
                                    op=Alu.add)
            psB = ps_sum.tile([1, 1], f32, tag="psB")
            nc.tensor.matmul(psB[:], bcol[:], ones_col[:], start=True, stop=True)

            # C = counts
            psC = ps_sum.tile([1, 1], f32, tag="psC")
            nc.tensor.matmul(psC[:], ccol[:], ones_col[:], start=True, stop=True)

            nc.vector.tensor_copy(results[:, 3 * k:3 * k + 1], psA[:])
            nc.vector.tensor_copy(results[:, 3 * k + 1:3 * k + 2], psB[:])
            nc.vector.tensor_copy(results[:, 3 * k + 2:3 * k + 3], psC[:])

        nc.sync.dma_start(outv[:], results[:])

    nc.finalize()
    return nc


_NC_CACHE = {}


def _get_module(reps=1):
    key = ("nc", reps)
    if key not in _NC_CACHE:
        _NC_CACHE[key] = _build_module(reps)
    return _NC_CACHE[key]


def _make_exec(nc):
    """Build a reusable jitted executor for the 8-core SPMD module.

    Mirrors concourse.bass2jax.run_bass_via_pjrt's multi-core branch but
    returns a callable so repeated executions reuse the compiled NEFF.
    """
    key = ("exec", id(nc))
    if key in _NC_CACHE:
        return _NC_CACHE[key]
    import jax
    import numpy as _np
    from jax.sharding import Mesh, PartitionSpec
    from jax.experimental.shard_map import shard_map
    from concourse import mybir
    from concourse import bass2jax as b2j

    b2j.install_neuronx_cc_hook()
    partition_name = (nc.partition_id_tensor.name
                      if nc.partition_id_tensor else None)
    in_names, out_names, out_avals, zero_outs = [], [], [], []
    for alloc in nc.m.functions[0].allocations:
        if not isinstance(alloc, mybir.MemoryLocationSet):
            continue
        name = alloc.memorylocations[0].name
        if alloc.kind == "ExternalInput":
            if name != partition_name:
                in_names.append(name)
        elif alloc.kind == "ExternalOutput":
            shape = tuple(alloc.tensor_shape)
            dtype = mybir.dt.np(alloc.dtype)
            out_names.append(name)
            out_avals.append(jax.core.ShapedArray(shape, dtype))
            zero_outs.append(_np.zeros(shape, dtype))
    n_params = len(in_names)
    n_outs = len(out_avals)
    all_in_names = tuple(in_names + out_names +
                         ([partition_name] if partition_name else []))
    donate = tuple(range(n_params, n_params + n_outs))

    def _body(*args):
        operands = list(args)
        if partition_name is not None:
            operands.append(b2j.partition_id_tensor())
        outs = b2j._bass_exec_p.bind(
            *operands,
            out_avals=tuple(out_avals),
            in_names=all_in_names,
            out_names=tuple(out_names),
            lowering_input_output_aliases=(),
            sim_require_finite=True,
            sim_require_nnan=True,
            nc=nc,
        )
        return tuple(outs)

    devices = jax.devices()[:N_CORES]
    mesh = Mesh(_np.asarray(devices), ("core",))
    in_specs = (PartitionSpec("core"),) * (n_params + n_outs)
    out_specs = (PartitionSpec("core"),) * n_outs
    sharded = jax.jit(
        shard_map(_body, mesh=mesh, in_specs=in_specs, out_specs=out_specs,
                  check_rep=False),
        donate_argnums=donate, keep_unused=True)

    def execute(in_maps, block=True):
        per_core = [[_np.asarray(m[name]) for name in in_names]
                    for m in in_maps]
        concat_in = [
            _np.concatenate([per_core[c][i] for c in range(N_CORES)], axis=0)
            for i in range(n_params)
        ]
        concat_zeros = [
            _np.zeros((N_CORES * z.shape[0], *z.shape[1:]), z.dtype)
            for z in zero_outs
        ]
        out_arrs = sharded(*concat_in, *concat_zeros)
        if block:
            jax.block_until_ready(out_arrs)
        return [
            {name: _np.asarray(out_arrs[i]).reshape(
                N_CORES, *out_avals[i].shape)[c]
             for i, name in enumerate(out_names)}
            for c in range(N_CORES)
        ]

    _NC_CACHE[key] = execute
    return execute


def _shard_inputs(bins, target_depth_maps):
    bins = np.ascontiguousarray(np.asarray(bins, dtype=np.float32)).reshape(2, 257, 16)
    tgt = np.ascontiguousarray(
        np.asarray(target_depth_maps, dtype=np.float32)).reshape(2, 448, 448)
    in_maps = []
    for c in range(N_CORES):
        ids = [4 * c + j for j in range(PATCHES_PER_CORE)]
        n = ids[0] // 16
        ls = [i % 16 for i in ids]
        bins4 = np.ascontiguousarray(bins[n][:, ls])           # (257, 4)
        blocks = []
        for l in ls:
            hb, wb = l // 4, l % 4
            blk = tgt[n, hb * 112:(hb + 1) * 112, wb * 112:(wb + 1) * 112]
            blocks.append(np.ascontiguousarray(blk).reshape(NPART, NT))
        pts = np.stack(blocks)                                  # (4, 128, 98)
        in_maps.append({"bins4": bins4, "pts": np.ascontiguousarray(pts)})
    return in_maps


def _combine(results):
    per_patch = []
    for c in range(N_CORES):
        vals = np.asarray(results[c]["outv"], dtype=np.float64).reshape(
            PATCHES_PER_CORE, 3)
        for k in range(PATCHES_PER_CORE):
            A, B, C = vals[k]
            if C > 0:
                per_patch.append(A / PC + B / max(C, 1.0))
            else:
                per_patch.append(0.0)
    return np.float32(np.mean(np.asarray(per_patch, dtype=np.float64)))


def run(inputs, reps=1):
    nc = _get_module(reps)
    execute = _make_exec(nc)
    in_maps = _shard_inputs(**inputs)
    results = execute(in_maps)
    val = _combine(results)
    return val, execute, in_maps


def kernel(**inputs) -> np.ndarray:
    val, _, _ = run(inputs)
    return np.array(val, dtype=np.float32)

